# revision 7
# baseline (speedup 1.0000x reference)
"""Trainium2 Bass kernel for the CustomRNN problem (v3).

Model (per batch element b):
    u_t = W_in @ x_t + bias + sigma*sqrt(2*alpha) * noise_t          [N=256]
    r_{t+1} = (1-alpha) * r_t + alpha * relu(W_rec @ r_t + u_t)
    out_t = W_out @ r_{t+1} + b_out                                  [3]

Sharding: data-parallel over batch across 8 cores (32 batch each), weights
replicated.

v3 design notes (vs v2): the run is latency-bound at ~553ns/step:
  L = STT(187) + sem(54) + 4 chain-matmul issues(~81) + PSUM pipe(175)
      + sem(54)
per step, with G=2 staggered 16-batch chains hiding each other's engine
work (total = T*L regardless of G; more groups would saturate the DVE).
v3 removes everything else from the PE's in-order path:
  - y-projection flipped: hist is the STATIONARY operand, W_out moves ->
    8 matmuls of 3 cols per block (24 cols) instead of 2x256 cols; output
    lands as [stepbatch-part, out] in PSUM, one small ACT copy per block.
  - drive matmuls split into 64-col halves; noise emitted 2 mms/step; ALL
    filler emission moved AFTER the chain ops of each step, so fillers
    never sit ahead of a waiting chain matmul in the in-order PE queue.
  - first chunk is 16 steps so the cold-start DMA is small; the tail only
    owes the last 2 steps' y work after the final state update.
Numerics identical to v2 (fp16 recurrence with exact fp16-decay rescaling,
fp8 x16 noise via identity matmuls, fp32 PSUM).
"""

import numpy as np

import concourse.bacc as bacc
import concourse.mybir as mybir
from concourse.tile import TileContext, add_dep_helper
from concourse.bass_utils import run_bass_kernel_spmd

ALPHA = 0.2
NOISE_SCALE = 0.05 * float(np.sqrt(2 * ALPHA))
DECAY = float(np.float16(1.0 - ALPHA))   # 0.7998046875, exact in fp16
N = 256
NCORES = 8
BC = 32          # batch per core
F16 = mybir.dt.float16
F32 = mybir.dt.float32
F8 = mybir.dt.float8e4      # e4m3
F8NP = mybir.dt.np(mybir.dt.float8e4)
NOISE_PREMUL = 16.0         # fp8 noise stored x16; identity diag = 1/16

_CACHE = {}


def _chunks(T, TC, first):
    """Chunk sizes: a small first chunk (cold-start DMA off the critical
    path), then TC-sized chunks, remainder absorbed at the end."""
    out = []
    o = 0
    if first and T > first:
        out.append((0, first))
        o = first
    while o < T:
        n = min(TC, T - o)
        out.append((o, n))
        o += n
    assert all(n % 8 == 0 for _, n in out)
    return out


def _build(T, TC, SB, G, first=16):
    GB = BC // G
    assert G * GB == BC and SB * 2 * GB * 4 <= 2048 and TC % SB == 0
    CHUNKS = _chunks(T, TC, first)
    NBLK_TOT = T // SB
    nc = bacc.Bacc("TRN2", num_devices=NCORES)

    noise_d = nc.dram_tensor("noiset", [128, T, 2 * BC], F8, kind="ExternalInput")
    xta_d = nc.dram_tensor("xta", [4, T, BC], F16, kind="ExternalInput")
    # all fp16 constants in one tensor (one DMA): w4 | win (4 rows) | wout
    # | block-0 xta (4 rows x SB*BC)
    cpk_d = nc.dram_tensor("cpack", [128, 774 + SB * BC], F16,
                           kind="ExternalInput")
    # fp8: identity/16 | block-0 noise (SB*2*BC cols)
    id_d = nc.dram_tensor("ident", [128, 128 + SB * 2 * BC], F8,
                          kind="ExternalInput")
    # y layout: [part, blk, col]: part = 32*tp + b, col = 3*p + o,
    # step = 8*blk + 2*p + tp, batch = part%32
    y_d = nc.dram_tensor("y", [64, NBLK_TOT, 12], F16, kind="ExternalOutput")

    with TileContext(nc) as tc:
        with (
            tc.tile_pool(name="consts", bufs=1) as consts,
            tc.tile_pool(name="hist", bufs=2) as histp,
            tc.tile_pool(name="noise", bufs=2) as noisep,
            tc.tile_pool(name="xtap", bufs=2) as xtap,
            tc.tile_pool(name="ysbp", bufs=2) as ysbp,
            tc.tile_pool(name="pv", bufs=3 * G, space="PSUM") as pvp,
            tc.tile_pool(name="pyp", bufs=2, space="PSUM") as pyp,
        ):
            cpk_sb = consts.tile_from(cpk_d[:, :])
            idp_sb = consts.tile_from(id_d[:, :])
            id_sb = idp_sb[:, 0:128]
            w4_sb = cpk_sb[:, 0:512]
            win_sb = cpk_sb[:, 512:768]
            wout_sb = cpk_sb[:, 768:774]
            xta0_sb = cpk_sb[0:4, 774:774 + SB * BC].rearrange(
                "p (t b) -> p t b", t=SB)
            noise0_sb = idp_sb[:, 128:128 + SB * 2 * BC].rearrange(
                "p (t c b) -> p t c b", t=SB, c=2)

            # Ordering-only (nosync) chain over every PE matmul: pins the
            # scheduler to the emission order.
            _prev_mm = [None]

            def mm(*args, **kw):
                inst = nc.tensor.matmul(*args, **kw)
                raw = getattr(inst, "ins", inst)
                if _prev_mm[0] is not None:
                    add_dep_helper(raw, _prev_mm[0], sync=False,
                                   reason="pe-stream-order")
                _prev_mm[0] = raw
                return inst

            nxt = None              # prefetched (noise_sb, xta_sb) for chunk+1
            carry_pvs = {}          # cross-chunk prefilled psum tiles
            carry_y = None          # (hist, b0, ysb, dma_info) deferred y-block
            prev_hist = None
            for ck, (ts0, TCk) in enumerate(CHUNKS):
                NBLK = TCk // SB
                gblk0 = ts0 // SB           # global block index of this chunk
                if nxt is None:
                    noise_sb = noisep.tile([128, TCk, 2 * BC], F8)
                    xta_sb = xtap.tile([4, TCk, BC], F16)
                    nc.sync.dma_start(out=xta_sb[:],
                                      in_=xta_d[:, ts0:ts0 + TCk, :])
                    nc.sync.dma_start(out=noise_sb[:],
                                      in_=noise_d[:, ts0:ts0 + TCk, :])
                else:
                    noise_sb, xta_sb = nxt
                if ck + 1 < len(CHUNKS):
                    nts0, nTC = CHUNKS[ck + 1]
                    n2 = noisep.tile([128, nTC, 2 * BC], F8, name="noise2")
                    nc.sync.dma_start(out=n2[:],
                                      in_=noise_d[:, nts0:nts0 + nTC, :])
                    x2 = xtap.tile([4, nTC, BC], F16, name="xta2")
                    nc.sync.dma_start(out=x2[:],
                                      in_=xta_d[:, nts0:nts0 + nTC, :])
                    nxt = (n2, x2)
                else:
                    nxt = None
                noise_r = noise_sb[:].rearrange("p t (c b) -> p t c b", c=2)
                noise_r2 = (nxt[0][:].rearrange("p t (c b) -> p t c b", c=2)
                            if nxt is not None else None)
                # hist layout [128, 2(k-chunk), (TCk+1)*BC]: slot s at
                # cols [s*BC, (s+1)*BC) holds state r_{ts0+s}; slot 0 =
                # carry-in.  Flat (t,b) so a 2-step slice is ONE free dim
                # (the y-projection stationary requires that).
                hist = histp.tile([128, 2, (TCk + 1) * BC], F16)
                ysb = ysbp.tile([64, NBLK, 12], F16)
                if ck == 0:
                    nc.vector.memset(hist[:, :, 0:BC], 0.0)

                pvs = carry_pvs
                carry_pvs = {}

                def emit_drive(key, b0, g, m_c, h, xt):
                    # one 64-col half: steps [b0+h*HB, b0+(h+1)*HB)
                    HB = SB // 2
                    gsl = slice(g * GB, (g + 1) * GB)
                    if (key, g) not in pvs:
                        pvs[(key, g)] = pvp.tile([128, SB, 2, GB], F32,
                                                 name="pv", tag="pv")
                    mm(pvs[(key, g)][:, h * HB:(h + 1) * HB, m_c, :],
                       win_sb[0:4, m_c * 128:(m_c + 1) * 128],
                       xt[:, b0 + h * HB:b0 + (h + 1) * HB, gsl],
                       start=(m_c == 0 and h == 0), stop=False,
                       skip_group_check=True)

                def emit_noise(key, b0, g, s0, s1, nr):
                    gsl = slice(g * GB, (g + 1) * GB)
                    for s in range(s0, s1):
                        mm(pvs[(key, g)][:, s], id_sb[:],
                           nr[:, b0 + s, :, gsl],
                           start=False, stop=False, skip_group_check=True)

                def emit_y_piece(key, b0, hist_, p):
                    # piece p: steps (2p, 2p+1) of block at b0 -> psum
                    # [64, 3p:3p+3]; stationary = hist slice (one flat
                    # 64-wide free dim), moving = wout (3 cols).  All pieces
                    # stay at partitions 0..63: matmul out partitions are
                    # pinned to the stationary's array columns, an out-AP
                    # partition offset would read stale columns.
                    if key not in pvs:
                        pvs[key] = pyp.tile([64, 12], F32, name="py", tag="py")
                    py = pvs[key]
                    h0 = (1 + b0 + 2 * p) * BC
                    for k_c in range(2):
                        mm(py[:, 3 * p:3 * p + 3],
                           hist_[:, k_c, h0:h0 + 2 * BC],
                           wout_sb[:, k_c * 3:(k_c + 1) * 3],
                           start=(p == 0 and k_c == 0), stop=(k_c == 1),
                           skip_group_check=True)

                def emit_y_copy(key, ysb_, blk_local):
                    py = pvs.pop(key)
                    nc.scalar.copy(ysb_[:, blk_local, :], py[:])

                for blk in range(NBLK):
                    b0 = blk * SB
                    if blk == 0 and (0, 0) not in pvs:
                        # cold start: prefill block 0 from const-packed copies
                        for g in range(G):
                            for h in range(2):
                                emit_drive(0, 0, g, 0, h, xta0_sb)
                                emit_drive(0, 0, g, 1, h, xta0_sb)
                        for g in range(G):
                            emit_noise(0, 0, g, 0, SB, noise0_sb)
                    # next prefill target: block blk+1, or next chunk's block 0
                    if blk + 1 < NBLK:
                        nkey, nb0, nxta, nnr = blk + 1, b0 + SB, xta_sb, noise_r
                    elif noise_r2 is not None:
                        nkey, nb0, nxta, nnr = "n0", 0, nxt[1], noise_r2
                    else:
                        nkey = None
                    last_chunk = noise_r2 is None
                    for s in range(SB):
                        l = b0 + s
                        # ---- chain ops first (PE seq head = chain mms) ----
                        if l == 0 and ck > 0:
                            rd, rs = prev_hist, prev_TC
                        else:
                            rd, rs = hist, l
                        for g in range(G):
                            g0 = g * GB
                            pv = pvs[(blk, g)]
                            for k_c in range(2):
                                for m_c in range(2):
                                    mm(pv[:, s, m_c],
                                       w4_sb[:, (2 * k_c + m_c) * 128:
                                             (2 * k_c + m_c + 1) * 128],
                                       rd[:, k_c,
                                          rs * BC + g0:rs * BC + g0 + GB],
                                       start=False, stop=(k_c == 1),
                                       skip_group_check=True)
                            # H' = max((1-a)*H, S1)  (single fused DVE op)
                            nc.vector.scalar_tensor_tensor(
                                out=hist[:, :, (l + 1) * BC + g0:
                                         (l + 1) * BC + g0 + GB],
                                in0=rd[:, :, rs * BC + g0:rs * BC + g0 + GB],
                                scalar=DECAY,
                                in1=pv[:, s],
                                op0=mybir.AluOpType.mult,
                                op1=mybir.AluOpType.max)
                        # ---- fillers after the chain (issue into the gap) --
                        # y for the previous block (or cross-chunk carry)
                        if s <= 4:
                            if blk > 0:
                                ykey, yb0, yhist = ("y",), b0 - SB, hist
                                yblk, yysb = blk - 1, ysb
                            elif carry_y is not None:
                                yhist, yb0, yysb, ydma = carry_y
                                ykey, yblk = ("yc",), None
                            else:
                                ykey = None
                            if ykey is not None:
                                if s < 4:
                                    emit_y_piece(ykey, yb0, yhist, s)
                                else:
                                    if yblk is None:
                                        # carry: copy into prev chunk's ysb
                                        py = pvs.pop(ykey)
                                        nb = yysb.shape[1]
                                        nc.scalar.copy(yysb[:, nb - 1, :],
                                                       py[:])
                                        nc.sync.dma_start(out=ydma[0],
                                                          in_=yysb[:])
                                        carry_y = None
                                    else:
                                        emit_y_copy(ykey, yysb, yblk)
                        # last chunk: also project THIS block's finished pairs
                        if last_chunk and blk == NBLK - 1 and s in (3, 5, 7):
                            p = (s - 3) // 2      # piece 0,1,2 (steps <= s-1)
                            emit_y_piece(("yf",), b0, hist, p)
                        # next-block prefill: drive 64-col pieces at a slot
                        # each (group starts first: s=0 g0, s=1 g1), noise
                        # pairs from s=2 (after both banks' start mms)
                        if nkey is not None:
                            g, m_c, h = s % 2, s // 4, (s // 2) % 2
                            emit_drive(nkey, nb0, g, m_c, h, nxta)
                            if 2 <= s < 6:
                                for g2 in range(G):
                                    emit_noise(nkey, nb0, g2, 2 * (s - 2),
                                               2 * (s - 2) + 2, nnr)
                    if blk > 0:
                        for g in range(G):
                            del pvs[(blk - 1, g)]
                if nxt is not None:
                    # defer this chunk's last y-block + its output DMA into
                    # the next chunk's filler slots (off the chain)
                    carry_y = (hist, (NBLK - 1) * SB, ysb,
                               (y_d[:, gblk0:gblk0 + NBLK, :],))
                else:
                    # final chunk: ship all but the last block early; after
                    # the loop only piece 3 + copy + small DMA remain
                    if NBLK > 1:
                        nc.sync.dma_start(
                            out=y_d[:, gblk0:gblk0 + NBLK - 1, :],
                            in_=ysb[:, 0:NBLK - 1])
                    emit_y_piece(("yf",), (NBLK - 1) * SB, hist, 3)
                    emit_y_copy(("yf",), ysb, NBLK - 1)
                    nc.sync.dma_start(
                        out=y_d[:, gblk0 + NBLK - 1:gblk0 + NBLK, :],
                        in_=ysb[:, NBLK - 1:NBLK])
                for g in range(G):
                    if ("n0", g) in pvs:
                        carry_pvs[(0, g)] = pvs.pop(("n0", g))
                prev_hist, prev_TC = hist, TCk
    nc.finalize()
    return nc


def get_nc(T=1000, TC=96, SB=8, G=2):
    key = (T, TC, SB, G)
    if key not in _CACHE:
        _CACHE[key] = _build(T, TC, SB, G)
    return _CACHE[key]


def make_inputs(x, noise, W_in, W_rec, W_out_w, W_out_b, bias):
    """Host-side shard + layout prep.  Returns in_maps for 8 cores.

    Exponential rescaling: the device recurrence uses decay d = fp16(0.8),
    slightly below the true 0.8.  Because relu is positively homogeneous,
    running the recurrence on r~_t = c^t r_t with c = d/0.8 (so 0.8*c = d
    exactly), drive scaled by c^(t+1), and the output rescaled by c^-(t+1)
    on the host reproduces the true-decay dynamics exactly.
    """
    x = np.asarray(x, np.float32)
    noise = np.asarray(noise, np.float32)
    W_in = np.asarray(W_in, np.float32)
    W_rec = np.asarray(W_rec, np.float32)
    W_out_w = np.asarray(W_out_w, np.float32)
    bias = np.asarray(bias, np.float32)
    B, T, _ = x.shape

    cfac = DECAY / (1.0 - ALPHA)                       # 0.99975586
    tfac = np.power(cfac, np.arange(1, T + 1), dtype=np.float64).astype(np.float32)

    cpack = np.zeros((128, 774 + 8 * BC), np.float16)  # w4|win|wout|xta blk0
    wrt = ALPHA * cfac * W_rec.T + DECAY * np.eye(256, dtype=np.float32)
    wrt = wrt.astype(np.float16)                       # [k, m]
    for k_c in range(2):
        for m_c in range(2):
            cpack[:, (2 * k_c + m_c) * 128:(2 * k_c + m_c + 1) * 128] = \
                wrt[128 * k_c:128 * (k_c + 1), 128 * m_c:128 * (m_c + 1)]
    ident = np.zeros((128, 128 + 8 * 2 * BC), F8NP)    # I/16 | noise blk0
    ident[:, 0:128] = (np.eye(128, dtype=np.float32) / NOISE_PREMUL).astype(F8NP)
    cpack[:3, 512:768] = (ALPHA * W_in.T).astype(np.float16)
    cpack[3, 512:768] = (ALPHA * bias).astype(np.float16)
    wt = np.asarray(W_out_w, np.float32).T.astype(np.float16)   # [n, 3]
    for k_c in range(2):
        cpack[:, 768 + 3 * k_c:768 + 3 * (k_c + 1)] = \
            wt[128 * k_c:128 * (k_c + 1)]

    nscale = ALPHA * NOISE_SCALE
    in_maps = []
    for c in range(NCORES):
        b0 = c * BC
        nz = (noise[b0:b0 + BC] * (NOISE_PREMUL * nscale * tfac[None, :, None])
              ).astype(F8NP)                           # [32, T, 256]
        nzt = np.ascontiguousarray(
            nz.reshape(BC, T, 2, 128).transpose(3, 1, 2, 0)).reshape(128, T, 2 * BC)
        xc = x[b0:b0 + BC] * tfac[None, :, None]       # [32, T, 3]
        xta = np.empty((4, T, BC), np.float16)
        xta[:3] = xc.transpose(2, 1, 0).astype(np.float16)
        xta[3] = tfac[:, None]
        cpk = cpack.copy()
        cpk[0:4, 774:774 + 8 * BC] = xta[:, 0:8, :].reshape(4, 8 * BC)
        idp = ident.copy()
        idp[:, 128:128 + 8 * 2 * BC] = nzt[:, 0:8, :].reshape(128, 8 * 2 * BC)
        in_maps.append({
            "noiset": nzt, "xta": xta, "cpack": cpk, "ident": idp,
        })
    return in_maps


def gather_output(results, B, T, W_out_b):
    cfac = DECAY / (1.0 - ALPHA)
    inv = np.power(cfac, -np.arange(1, T + 1), dtype=np.float64).astype(np.float32)
    out = np.empty((B, T, 3), np.float32)
    nblk = T // 8
    for c in range(NCORES):
        y = results[c]["y"].astype(np.float32)         # [64, nblk, 12]
        y = y.reshape(2, BC, nblk, 4, 3)               # [tp, b, blk, p, o]
        # step = 8*blk + 2*p + tp
        y = y.transpose(1, 2, 3, 0, 4)                 # [b, blk, p, tp, o]
        out[c * BC:(c + 1) * BC] = y.reshape(BC, T, 3)
    out *= inv[None, :, None]
    out += np.asarray(W_out_b, np.float32)[None, None, :]
    return out


def kernel(x, noise, W_in, W_rec, W_out_w, W_out_b, bias):
    x = np.asarray(x, np.float32)
    B, T, _ = x.shape
    nc = get_nc(T=T)
    in_maps = make_inputs(x, noise, W_in, W_rec, W_out_w, W_out_b, bias)
    res = run_bass_kernel_spmd(nc, in_maps, list(range(NCORES)))
    return gather_output(res.results, B, T, W_out_b)


# revision 8
# speedup vs baseline: 1.2048x; 1.2048x over previous
"""Trainium2 Bass kernel for the CustomRNN problem (v3).

Model (per batch element b):
    u_t = W_in @ x_t + bias + sigma*sqrt(2*alpha) * noise_t          [N=256]
    r_{t+1} = (1-alpha) * r_t + alpha * relu(W_rec @ r_t + u_t)
    out_t = W_out @ r_{t+1} + b_out                                  [3]

Sharding: data-parallel over batch across 8 cores (32 batch each), weights
replicated.

v3 design notes (vs v2): the run is latency-bound at ~553ns/step:
  L = STT(187) + sem(54) + 4 chain-matmul issues(~81) + PSUM pipe(175)
      + sem(54)
per step, with G=2 staggered 16-batch chains hiding each other's engine
work (total = T*L regardless of G; more groups would saturate the DVE).
v3 removes everything else from the PE's in-order path:
  - y-projection flipped: hist is the STATIONARY operand, W_out moves ->
    8 matmuls of 3 cols per block (24 cols) instead of 2x256 cols; output
    lands as [stepbatch-part, out] in PSUM, one small ACT copy per block.
  - drive matmuls split into 64-col halves; noise emitted 2 mms/step; ALL
    filler emission moved AFTER the chain ops of each step, so fillers
    never sit ahead of a waiting chain matmul in the in-order PE queue.
  - first chunk is 16 steps so the cold-start DMA is small; the tail only
    owes the last 2 steps' y work after the final state update.
Numerics identical to v2 (fp16 recurrence with exact fp16-decay rescaling,
fp8 x16 noise via identity matmuls, fp32 PSUM).
"""

import numpy as np

import concourse.bacc as bacc
import concourse.mybir as mybir
from concourse.tile import TileContext, add_dep_helper
from concourse.bass_utils import run_bass_kernel_spmd

ALPHA = 0.2
NOISE_SCALE = 0.05 * float(np.sqrt(2 * ALPHA))
DECAY = float(np.float16(1.0 - ALPHA))   # 0.7998046875, exact in fp16
N = 256
NCORES = 8
BC = 32          # batch per core
F16 = mybir.dt.float16
F32 = mybir.dt.float32
F8 = mybir.dt.float8e4      # e4m3
F8NP = mybir.dt.np(mybir.dt.float8e4)
NOISE_PREMUL = 16.0         # fp8 noise stored x16; identity diag = 1/16

_CACHE = {}


def _chunks(T, TC, first):
    """Chunk sizes: a small first chunk (cold-start DMA off the critical
    path), then TC-sized chunks, remainder absorbed at the end."""
    out = []
    o = 0
    if first and T > first:
        out.append((0, first))
        o = first
    while o < T:
        n = min(TC, T - o)
        out.append((o, n))
        o += n
    assert all(n % 8 == 0 for _, n in out)
    return out


def _build(T, TC, SB, G, first=16):
    GB = BC // G
    assert G * GB == BC and SB * 2 * GB * 4 <= 2048 and TC % SB == 0
    CHUNKS = _chunks(T, TC, first)
    NBLK_TOT = T // SB
    nc = bacc.Bacc("TRN2", num_devices=NCORES)

    noise_d = nc.dram_tensor("noiset", [128, T, 2 * BC], F8, kind="ExternalInput")
    xta_d = nc.dram_tensor("xta", [4, T, BC], F16, kind="ExternalInput")
    # all fp16 constants in one tensor (one DMA): w4 | win (4 rows) | wout
    # | block-0 xta (4 rows x SB*BC)
    cpk_d = nc.dram_tensor("cpack", [128, 774 + SB * BC], F16,
                           kind="ExternalInput")
    # fp8: identity/16 | block-0 noise (SB*2*BC cols)
    id_d = nc.dram_tensor("ident", [128, 128 + SB * 2 * BC], F8,
                          kind="ExternalInput")
    y_d = nc.dram_tensor("y", [3, T, BC], F16, kind="ExternalOutput")

    with TileContext(nc) as tc:
        with (
            tc.tile_pool(name="consts", bufs=1) as consts,
            tc.tile_pool(name="hist", bufs=2) as histp,
            tc.tile_pool(name="noise", bufs=2) as noisep,
            tc.tile_pool(name="xtap", bufs=2) as xtap,
            tc.tile_pool(name="ysbp", bufs=2) as ysbp,
            tc.tile_pool(name="pv", bufs=3 * G, space="PSUM") as pvp,
            tc.tile_pool(name="pyp", bufs=2, space="PSUM") as pyp,
        ):
            cpk_sb = consts.tile_from(cpk_d[:, :])
            idp_sb = consts.tile_from(id_d[:, :])
            id_sb = idp_sb[:, 0:128]
            w4_sb = cpk_sb[:, 0:512]
            win_sb = cpk_sb[:, 512:768]
            wout_sb = cpk_sb[:, 768:774]
            xta0_sb = cpk_sb[0:4, 774:774 + SB * BC].rearrange(
                "p (t b) -> p t b", t=SB)
            noise0_sb = idp_sb[:, 128:128 + SB * 2 * BC].rearrange(
                "p (t c b) -> p t c b", t=SB, c=2)

            # Ordering-only (nosync) chain over every PE matmul: pins the
            # scheduler to the emission order.
            _prev_mm = [None]

            def mm(*args, **kw):
                inst = nc.tensor.matmul(*args, **kw)
                raw = getattr(inst, "ins", inst)
                if _prev_mm[0] is not None:
                    add_dep_helper(raw, _prev_mm[0], sync=False,
                                   reason="pe-stream-order")
                _prev_mm[0] = raw
                return inst

            nxt = None              # prefetched (noise_sb, xta_sb) for chunk+1
            carry_pvs = {}          # cross-chunk prefilled psum tiles
            carry_y = None          # (hist, b0, ysb, dma_info) deferred y-block
            prev_hist = None
            for ck, (ts0, TCk) in enumerate(CHUNKS):
                NBLK = TCk // SB
                gblk0 = ts0 // SB           # global block index of this chunk
                if nxt is None:
                    noise_sb = noisep.tile([128, TCk, 2 * BC], F8)
                    xta_sb = xtap.tile([4, TCk, BC], F16)
                    nc.sync.dma_start(out=xta_sb[:],
                                      in_=xta_d[:, ts0:ts0 + TCk, :])
                    nc.sync.dma_start(out=noise_sb[:],
                                      in_=noise_d[:, ts0:ts0 + TCk, :])
                else:
                    noise_sb, xta_sb = nxt
                if ck + 1 < len(CHUNKS):
                    nts0, nTC = CHUNKS[ck + 1]
                    n2 = noisep.tile([128, nTC, 2 * BC], F8, name="noise2")
                    nc.sync.dma_start(out=n2[:],
                                      in_=noise_d[:, nts0:nts0 + nTC, :])
                    x2 = xtap.tile([4, nTC, BC], F16, name="xta2")
                    nc.sync.dma_start(out=x2[:],
                                      in_=xta_d[:, nts0:nts0 + nTC, :])
                    nxt = (n2, x2)
                else:
                    nxt = None
                noise_r = noise_sb[:].rearrange("p t (c b) -> p t c b", c=2)
                noise_r2 = (nxt[0][:].rearrange("p t (c b) -> p t c b", c=2)
                            if nxt is not None else None)
                # hist slot s holds state r_{ts0+s}; slot 0 = carry-in
                hist = histp.tile([128, TCk + 1, 2, BC], F16)
                ysb = ysbp.tile([3, TCk, BC], F16)
                if ck == 0:
                    nc.vector.memset(hist[:, 0], 0.0)

                pvs = carry_pvs
                carry_pvs = {}

                def emit_drive(key, b0, g, m_c, h, xt):
                    # one 64-col half: steps [b0+h*HB, b0+(h+1)*HB)
                    HB = SB // 2
                    gsl = slice(g * GB, (g + 1) * GB)
                    if (key, g) not in pvs:
                        pvs[(key, g)] = pvp.tile([128, SB, 2, GB], F32,
                                                 name="pv", tag="pv")
                    mm(pvs[(key, g)][:, h * HB:(h + 1) * HB, m_c, :],
                       win_sb[0:4, m_c * 128:(m_c + 1) * 128],
                       xt[:, b0 + h * HB:b0 + (h + 1) * HB, gsl],
                       start=(m_c == 0 and h == 0), stop=False,
                       skip_group_check=True)

                def emit_noise(key, b0, g, s0, s1, nr):
                    gsl = slice(g * GB, (g + 1) * GB)
                    for s in range(s0, s1):
                        mm(pvs[(key, g)][:, s], id_sb[:],
                           nr[:, b0 + s, :, gsl],
                           start=False, stop=False, skip_group_check=True)

                def emit_y_piece(key, b0, hist_, p):
                    # piece p: steps (2p, 2p+1) of block at b0; stationary =
                    # wout chunk (tiny LDW), moving = hist 2-step slice (64
                    # cols) -> keeps the array warm without 213ns monoliths.
                    if key not in pvs:
                        pvs[key] = pyp.tile([3, SB, BC], F32,
                                            name="py", tag="py")
                    py = pvs[key]
                    for k_c in range(2):
                        mm(py[:, 2 * p:2 * p + 2, :],
                           wout_sb[:, k_c * 3:(k_c + 1) * 3],
                           hist_[:, 1 + b0 + 2 * p:1 + b0 + 2 * p + 2,
                                 k_c, :],
                           start=(p == 0 and k_c == 0), stop=(k_c == 1),
                           skip_group_check=True)

                def emit_y_copy(key, ysb_, b0_local):
                    py = pvs.pop(key)
                    nc.scalar.copy(ysb_[:, b0_local:b0_local + SB, :], py[:])

                for blk in range(NBLK):
                    b0 = blk * SB
                    if blk == 0 and (0, 0) not in pvs:
                        # cold start: prefill block 0 from const-packed copies
                        for g in range(G):
                            for h in range(2):
                                emit_drive(0, 0, g, 0, h, xta0_sb)
                                emit_drive(0, 0, g, 1, h, xta0_sb)
                        for g in range(G):
                            emit_noise(0, 0, g, 0, SB, noise0_sb)
                    # next prefill target: block blk+1, or next chunk's block 0
                    if blk + 1 < NBLK:
                        nkey, nb0, nxta, nnr = blk + 1, b0 + SB, xta_sb, noise_r
                    elif noise_r2 is not None:
                        nkey, nb0, nxta, nnr = "n0", 0, nxt[1], noise_r2
                    else:
                        nkey = None
                    last_chunk = noise_r2 is None
                    for s in range(SB):
                        l = b0 + s
                        # ---- chain ops first (PE seq head = chain mms) ----
                        if l == 0 and ck > 0:
                            rd, rs = prev_hist, prev_TC
                        else:
                            rd, rs = hist, l
                        for g in range(G):
                            gsl = slice(g * GB, (g + 1) * GB)
                            pv = pvs[(blk, g)]
                            for k_c in range(2):
                                for m_c in range(2):
                                    mm(pv[:, s, m_c],
                                       w4_sb[:, (2 * k_c + m_c) * 128:
                                             (2 * k_c + m_c + 1) * 128],
                                       rd[:, rs, k_c, gsl],
                                       start=False, stop=(k_c == 1),
                                       skip_group_check=True)
                            # H' = max((1-a)*H, S1)  (single fused DVE op)
                            nc.vector.scalar_tensor_tensor(
                                out=hist[:, l + 1, :, gsl],
                                in0=rd[:, rs, :, gsl],
                                scalar=DECAY,
                                in1=pv[:, s],
                                op0=mybir.AluOpType.mult,
                                op1=mybir.AluOpType.max)
                        # ---- fillers after the chain (issue into the gap) --
                        # y for the previous block (or cross-chunk carry)
                        if s <= 4:
                            if blk > 0:
                                ykey, yb0, yhist = ("y",), b0 - SB, hist
                                yblk, yysb = blk - 1, ysb
                            elif carry_y is not None:
                                yhist, yb0, yysb, ydma = carry_y
                                ykey, yblk = ("yc",), None
                            else:
                                ykey = None
                            if ykey is not None:
                                if s < 4:
                                    emit_y_piece(ykey, yb0, yhist, s)
                                else:
                                    if yblk is None:
                                        # carry: copy into prev chunk's ysb
                                        py = pvs.pop(ykey)
                                        nb = yysb.shape[1]
                                        nc.scalar.copy(
                                            yysb[:, nb - SB:nb, :], py[:])
                                        nc.sync.dma_start(out=ydma[0],
                                                          in_=yysb[:])
                                        carry_y = None
                                    else:
                                        emit_y_copy(ykey, yysb, yblk * SB)
                        # last chunk: also project THIS block's finished pairs
                        if last_chunk and blk == NBLK - 1 and s in (3, 5, 7):
                            p = (s - 3) // 2      # piece 0,1,2 (steps <= s-1)
                            emit_y_piece(("yf",), b0, hist, p)
                        # next-block prefill: drive 64-col pieces at a slot
                        # each (group starts first: s=0 g0, s=1 g1), noise
                        # pairs from s=2 (after both banks' start mms)
                        if nkey is not None:
                            g, m_c, h = s % 2, s // 4, (s // 2) % 2
                            emit_drive(nkey, nb0, g, m_c, h, nxta)
                            if 2 <= s < 6:
                                for g2 in range(G):
                                    emit_noise(nkey, nb0, g2, 2 * (s - 2),
                                               2 * (s - 2) + 2, nnr)
                    if blk > 0:
                        for g in range(G):
                            del pvs[(blk - 1, g)]
                if nxt is not None:
                    # defer this chunk's last y-block + its output DMA into
                    # the next chunk's filler slots (off the chain)
                    carry_y = (hist, (NBLK - 1) * SB, ysb,
                               (y_d[:, ts0:ts0 + TCk, :],))
                else:
                    # final chunk: ship all but the last block early; after
                    # the loop only piece 3 + copy + small DMA remain
                    lb0 = (NBLK - 1) * SB
                    if NBLK > 1:
                        nc.sync.dma_start(out=y_d[:, ts0:ts0 + lb0, :],
                                          in_=ysb[:, 0:lb0])
                    emit_y_piece(("yf",), lb0, hist, 3)
                    emit_y_copy(("yf",), ysb, lb0)
                    nc.sync.dma_start(out=y_d[:, ts0 + lb0:ts0 + TCk, :],
                                      in_=ysb[:, lb0:])
                for g in range(G):
                    if ("n0", g) in pvs:
                        carry_pvs[(0, g)] = pvs.pop(("n0", g))
                prev_hist, prev_TC = hist, TCk
    nc.finalize()
    return nc


def get_nc(T=1000, TC=96, SB=8, G=2):
    key = (T, TC, SB, G)
    if key not in _CACHE:
        _CACHE[key] = _build(T, TC, SB, G)
    return _CACHE[key]


def make_inputs(x, noise, W_in, W_rec, W_out_w, W_out_b, bias):
    """Host-side shard + layout prep.  Returns in_maps for 8 cores.

    Exponential rescaling: the device recurrence uses decay d = fp16(0.8),
    slightly below the true 0.8.  Because relu is positively homogeneous,
    running the recurrence on r~_t = c^t r_t with c = d/0.8 (so 0.8*c = d
    exactly), drive scaled by c^(t+1), and the output rescaled by c^-(t+1)
    on the host reproduces the true-decay dynamics exactly.
    """
    x = np.asarray(x, np.float32)
    noise = np.asarray(noise, np.float32)
    W_in = np.asarray(W_in, np.float32)
    W_rec = np.asarray(W_rec, np.float32)
    W_out_w = np.asarray(W_out_w, np.float32)
    bias = np.asarray(bias, np.float32)
    B, T, _ = x.shape

    cfac = DECAY / (1.0 - ALPHA)                       # 0.99975586
    tfac = np.power(cfac, np.arange(1, T + 1), dtype=np.float64).astype(np.float32)

    cpack = np.zeros((128, 774 + 8 * BC), np.float16)  # w4|win|wout|xta blk0
    wrt = ALPHA * cfac * W_rec.T + DECAY * np.eye(256, dtype=np.float32)
    wrt = wrt.astype(np.float16)                       # [k, m]
    for k_c in range(2):
        for m_c in range(2):
            cpack[:, (2 * k_c + m_c) * 128:(2 * k_c + m_c + 1) * 128] = \
                wrt[128 * k_c:128 * (k_c + 1), 128 * m_c:128 * (m_c + 1)]
    ident = np.zeros((128, 128 + 8 * 2 * BC), F8NP)    # I/16 | noise blk0
    ident[:, 0:128] = (np.eye(128, dtype=np.float32) / NOISE_PREMUL).astype(F8NP)
    cpack[:3, 512:768] = (ALPHA * W_in.T).astype(np.float16)
    cpack[3, 512:768] = (ALPHA * bias).astype(np.float16)
    wt = np.asarray(W_out_w, np.float32).T.astype(np.float16)   # [n, 3]
    for k_c in range(2):
        cpack[:, 768 + 3 * k_c:768 + 3 * (k_c + 1)] = \
            wt[128 * k_c:128 * (k_c + 1)]

    nscale = ALPHA * NOISE_SCALE
    in_maps = []
    for c in range(NCORES):
        b0 = c * BC
        nz = (noise[b0:b0 + BC] * (NOISE_PREMUL * nscale * tfac[None, :, None])
              ).astype(F8NP)                           # [32, T, 256]
        nzt = np.ascontiguousarray(
            nz.reshape(BC, T, 2, 128).transpose(3, 1, 2, 0)).reshape(128, T, 2 * BC)
        xc = x[b0:b0 + BC] * tfac[None, :, None]       # [32, T, 3]
        xta = np.empty((4, T, BC), np.float16)
        xta[:3] = xc.transpose(2, 1, 0).astype(np.float16)
        xta[3] = tfac[:, None]
        cpk = cpack.copy()
        cpk[0:4, 774:774 + 8 * BC] = xta[:, 0:8, :].reshape(4, 8 * BC)
        idp = ident.copy()
        idp[:, 128:128 + 8 * 2 * BC] = nzt[:, 0:8, :].reshape(128, 8 * 2 * BC)
        in_maps.append({
            "noiset": nzt, "xta": xta, "cpack": cpk, "ident": idp,
        })
    return in_maps


def gather_output(results, B, T, W_out_b):
    cfac = DECAY / (1.0 - ALPHA)
    inv = np.power(cfac, -np.arange(1, T + 1), dtype=np.float64).astype(np.float32)
    out = np.empty((B, T, 3), np.float32)
    for c in range(NCORES):
        out[c * BC:(c + 1) * BC] = \
            results[c]["y"].transpose(2, 1, 0).astype(np.float32)
    out *= inv[None, :, None]
    out += np.asarray(W_out_b, np.float32)[None, None, :]
    return out


def kernel(x, noise, W_in, W_rec, W_out_w, W_out_b, bias):
    x = np.asarray(x, np.float32)
    B, T, _ = x.shape
    nc = get_nc(T=T)
    in_maps = make_inputs(x, noise, W_in, W_rec, W_out_w, W_out_b, bias)
    res = run_bass_kernel_spmd(nc, in_maps, list(range(NCORES)))
    return gather_output(res.results, B, T, W_out_b)


# revision 10
# speedup vs baseline: 1.3260x; 1.1007x over previous
"""Trainium2 Bass kernel for the CustomRNN problem (v5).

Model (per batch element b):
    u_t = W_in @ x_t + bias + sigma*sqrt(2*alpha) * noise_t          [N=256]
    r_{t+1} = (1-alpha) * r_t + alpha * relu(W_rec @ r_t + u_t)
    out_t = W_out @ r_{t+1} + b_out                                  [3]

Sharding: data-parallel over batch across 8 cores (32 batch each), weights
replicated.

v5 design notes: the run is latency-bound by the per-step serial cycle
    STT(DVE) -> sem -> 4 chain matmuls -> PSUM pipe -> sem -> STT'
with G=2 staggered 16-batch chains.  In v2..v4 the two groups' state
updates wrote ONE hist tile, so each group's chain matmuls waited on BOTH
groups' DVE ops (coarse region dep + counting semaphore) — putting
STT_A+STT_B (~240ns) on the critical path.  v5 gives each group its OWN
hist tile, so group A's cycle only contains its own STT: predicted cycle
~470ns instead of ~553ns.
Filler work (drive/noise prefill for the next block, y projection of the
previous block) is emitted at the top of designated steps, one filler
TYPE per slot (mixed-stationary filler runs pay ~150ns boundary holes),
each piece <= 128 cols so it never monopolizes the array (the in-order PE
queue turns long fillers into chain stalls).  Keep total PE array duty
comparable to v2 — dropping it demotes the PE p-state and slows the chain
(observed v3: MID->LOW pstate, everything x1.5).
Numerics identical to v2 (fp16 recurrence with exact fp16-decay rescaling,
fp8 x16 noise via identity matmuls, fp32 PSUM).
"""

import numpy as np

import concourse.bacc as bacc
import concourse.mybir as mybir
from concourse.tile import TileContext, add_dep_helper
from concourse.bass_utils import run_bass_kernel_spmd

ALPHA = 0.2
NOISE_SCALE = 0.05 * float(np.sqrt(2 * ALPHA))
DECAY = float(np.float16(1.0 - ALPHA))   # 0.7998046875, exact in fp16
N = 256
NCORES = 8
BC = 32          # batch per core
F16 = mybir.dt.float16
F32 = mybir.dt.float32
F8 = mybir.dt.float8e4      # e4m3
F8NP = mybir.dt.np(mybir.dt.float8e4)
NOISE_PREMUL = 16.0         # fp8 noise stored x16; identity diag = 1/16

_CACHE = {}


def _chunks(T, TC, first):
    """Chunk sizes: a small first chunk (cold-start DMA off the critical
    path), then TC-sized chunks, remainder absorbed at the end."""
    out = []
    o = 0
    if first and T > first:
        out.append((0, first))
        o = first
    while o < T:
        n = min(TC, T - o)
        out.append((o, n))
        o += n
    assert all(n % 8 == 0 for _, n in out)
    return out


def _build(T, TC, SB, G, first=16):
    GB = BC // G
    assert G * GB == BC and SB * 2 * GB * 4 <= 2048 and TC % SB == 0
    CHUNKS = _chunks(T, TC, first)
    nc = bacc.Bacc("TRN2", num_devices=NCORES)

    noise_d = nc.dram_tensor("noiset", [128, T, 2 * BC], F8, kind="ExternalInput")
    xta_d = nc.dram_tensor("xta", [4, T, BC], F16, kind="ExternalInput")
    # all fp16 constants in one tensor (one DMA): w4 | win (4 rows) | wout
    # | block-0 xta (4 rows x SB*BC)
    cpk_d = nc.dram_tensor("cpack", [128, 774 + SB * BC], F16,
                           kind="ExternalInput")
    # fp8: identity/16 | block-0 noise (SB*2*BC cols)
    id_d = nc.dram_tensor("ident", [128, 128 + SB * 2 * BC], F8,
                          kind="ExternalInput")
    y_d = nc.dram_tensor("y", [3, T, BC], F16, kind="ExternalOutput")

    with TileContext(nc) as tc:
        with (
            tc.tile_pool(name="consts", bufs=1) as consts,
            tc.tile_pool(name="hist", bufs=2 * G) as histp,
            tc.tile_pool(name="noise", bufs=2) as noisep,
            tc.tile_pool(name="xtap", bufs=2) as xtap,
            tc.tile_pool(name="ysbp", bufs=2) as ysbp,
            tc.tile_pool(name="pv", bufs=3 * G, space="PSUM") as pvp,
            tc.tile_pool(name="pyp", bufs=2, space="PSUM") as pyp,
        ):
            cpk_sb = consts.tile_from(cpk_d[:, :])
            idp_sb = consts.tile_from(id_d[:, :])
            id_sb = idp_sb[:, 0:128]
            w4_sb = cpk_sb[:, 0:512]
            win_sb = cpk_sb[:, 512:768]
            wout_sb = cpk_sb[:, 768:774]
            xta0_sb = cpk_sb[0:4, 774:774 + SB * BC].rearrange(
                "p (t b) -> p t b", t=SB)
            noise0_sb = idp_sb[:, 128:128 + SB * 2 * BC].rearrange(
                "p (t c b) -> p t c b", t=SB, c=2)

            # Ordering-only (nosync) chain over every PE matmul: pins the
            # scheduler to the emission order.
            _prev_mm = [None]

            def mm(*args, **kw):
                inst = nc.tensor.matmul(*args, **kw)
                raw = getattr(inst, "ins", inst)
                if _prev_mm[0] is not None:
                    add_dep_helper(raw, _prev_mm[0], sync=False,
                                   reason="pe-stream-order")
                _prev_mm[0] = raw
                return inst

            nxt = None              # prefetched (noise_sb, xta_sb) for chunk+1
            carry_pvs = {}          # cross-chunk prefilled psum tiles
            carry_y = None          # (hists, b0, ysb, dma) deferred y-block
            prev_hists = None
            for ck, (ts0, TCk) in enumerate(CHUNKS):
                NBLK = TCk // SB
                if nxt is None:
                    noise_sb = noisep.tile([128, TCk, 2 * BC], F8)
                    xta_sb = xtap.tile([4, TCk, BC], F16)
                    nc.sync.dma_start(out=xta_sb[:],
                                      in_=xta_d[:, ts0:ts0 + TCk, :])
                    nc.sync.dma_start(out=noise_sb[:],
                                      in_=noise_d[:, ts0:ts0 + TCk, :])
                else:
                    noise_sb, xta_sb = nxt
                if ck + 1 < len(CHUNKS):
                    nts0, nTC = CHUNKS[ck + 1]
                    n2 = noisep.tile([128, nTC, 2 * BC], F8, name="noise2")
                    nc.sync.dma_start(out=n2[:],
                                      in_=noise_d[:, nts0:nts0 + nTC, :])
                    x2 = xtap.tile([4, nTC, BC], F16, name="xta2")
                    nc.sync.dma_start(out=x2[:],
                                      in_=xta_d[:, nts0:nts0 + nTC, :])
                    nxt = (n2, x2)
                else:
                    nxt = None
                noise_r = noise_sb[:].rearrange("p t (c b) -> p t c b", c=2)
                noise_r2 = (nxt[0][:].rearrange("p t (c b) -> p t c b", c=2)
                            if nxt is not None else None)
                # per-GROUP hist tiles: slot s holds that group's state
                # r_{ts0+s}; slot 0 = carry-in.  Separate tiles per group so
                # a group's chain matmuls wait only on its OWN state update.
                hists = [histp.tile([128, TCk + 1, 2, GB], F16,
                                    name=f"hist{g}") for g in range(G)]
                ysb = ysbp.tile([3, TCk, BC], F16)
                if ck == 0:
                    for g in range(G):
                        nc.vector.memset(hists[g][:, 0], 0.0)

                pvs = carry_pvs
                carry_pvs = {}

                def emit_drive(key, b0, g, m_c, xt):
                    # PSUM bank protocol: exactly one start=True per bank.
                    gsl = slice(g * GB, (g + 1) * GB)
                    if (key, g) not in pvs:
                        pvs[(key, g)] = pvp.tile([128, SB, 2, GB], F32,
                                                 name="pv", tag="pv")
                    mm(pvs[(key, g)][:, :, m_c, :],
                       win_sb[0:4, m_c * 128:(m_c + 1) * 128],
                       xt[:, b0:b0 + SB, gsl],
                       start=(m_c == 0), stop=False, skip_group_check=True)

                def emit_noise(key, b0, g, s0, s1, nr):
                    gsl = slice(g * GB, (g + 1) * GB)
                    for s in range(s0, s1):
                        mm(pvs[(key, g)][:, s], id_sb[:],
                           nr[:, b0 + s, :, gsl],
                           start=False, stop=False, skip_group_check=True)

                def emit_y_half(key, b0, hists_, k_c, s0=0, s1=SB):
                    # y += W_out[k-chunk]^T @ r for steps [b0+s0, b0+s1),
                    # one matmul per group ((s1-s0)*GB cols each).
                    if key not in pvs:
                        pvs[key] = pyp.tile([3, SB, BC], F32,
                                            name="py", tag="py")
                    py = pvs[key]
                    first = (key, "st") not in pvs
                    pvs[(key, "st")] = True
                    for g in range(G):
                        gsl = slice(g * GB, (g + 1) * GB)
                        mm(py[:, s0:s1, gsl],
                           wout_sb[:, k_c * 3:(k_c + 1) * 3],
                           hists_[g][:, 1 + b0 + s0:1 + b0 + s1, k_c, :],
                           start=(first and g == 0), stop=(k_c == 1),
                           skip_group_check=True)

                def emit_y_copy(key, ysb_, b0_local):
                    pvs.pop((key, "st"), None)
                    py = pvs.pop(key)
                    nc.scalar.copy(ysb_[:, b0_local:b0_local + SB, :], py[:])

                for blk in range(NBLK):
                    b0 = blk * SB
                    if blk == 0 and (0, 0) not in pvs:
                        # cold start: prefill block 0 from const-packed copies
                        for g in range(G):
                            emit_drive(0, 0, g, 0, xta0_sb)
                            emit_drive(0, 0, g, 1, xta0_sb)
                        for g in range(G):
                            emit_noise(0, 0, g, 0, SB, noise0_sb)
                    # next prefill target: block blk+1, or next chunk's block 0
                    if blk + 1 < NBLK:
                        nkey, nb0, nxta, nnr = blk + 1, b0 + SB, xta_sb, noise_r
                    elif noise_r2 is not None:
                        nkey, nb0, nxta, nnr = "n0", 0, nxt[1], noise_r2
                    else:
                        nkey = None
                    last_blk = noise_r2 is None and blk == NBLK - 1
                    for s in range(SB):
                        l = b0 + s
                        # ---- fillers first (issue during the chain wait,
                        #      one filler type per slot) ----
                        if s == 1 or s == 2:
                            k_c = s - 1
                            if blk > 0:
                                emit_y_half(("y",), b0 - SB, hists, k_c)
                                if s == 2:
                                    emit_y_copy(("y",), ysb, b0 - SB)
                            elif carry_y is not None:
                                chists, cb0, cysb, cdma = carry_y
                                emit_y_half(("yc",), cb0, chists, k_c)
                                if s == 2:
                                    nb = cysb.shape[1]
                                    pvs.pop((("yc",), "st"), None)
                                    py = pvs.pop(("yc",))
                                    nc.scalar.copy(cysb[:, nb - SB:nb, :],
                                                   py[:])
                                    nc.sync.dma_start(out=cdma, in_=cysb[:])
                                    carry_y = None
                        elif s == 3 and nkey is not None:
                            for g in range(G):
                                emit_drive(nkey, nb0, g, 0, nxta)
                        elif s == 4 and nkey is not None:
                            for g in range(G):
                                emit_drive(nkey, nb0, g, 1, nxta)
                        elif s == 5 and nkey is not None:
                            emit_noise(nkey, nb0, 0, 0, SB, nnr)
                        elif s == 6 and nkey is not None and G > 1:
                            emit_noise(nkey, nb0, 1, 0, SB, nnr)
                        elif s == 5 and last_blk:
                            # final block: project finished steps 0..3 early
                            emit_y_half(("yf",), b0, hists, 0, 0, SB // 2)
                            emit_y_half(("yf",), b0, hists, 1, 0, SB // 2)
                        # ---- chain ----
                        if l == 0 and ck > 0:
                            rds, rs = prev_hists, prev_TC
                        else:
                            rds, rs = hists, l
                        for g in range(G):
                            pv = pvs[(blk, g)]
                            for k_c in range(2):
                                for m_c in range(2):
                                    mm(pv[:, s, m_c],
                                       w4_sb[:, (2 * k_c + m_c) * 128:
                                             (2 * k_c + m_c + 1) * 128],
                                       rds[g][:, rs, k_c, :],
                                       start=False, stop=(k_c == 1),
                                       skip_group_check=True)
                            # H' = max((1-a)*H, S1)  (single fused DVE op)
                            nc.vector.scalar_tensor_tensor(
                                out=hists[g][:, l + 1, :, :],
                                in0=rds[g][:, rs, :, :],
                                scalar=DECAY,
                                in1=pv[:, s],
                                op0=mybir.AluOpType.mult,
                                op1=mybir.AluOpType.max)
                    if blk > 0:
                        for g in range(G):
                            del pvs[(blk - 1, g)]
                if nxt is not None:
                    # defer this chunk's last y-block + its output DMA into
                    # the next chunk's filler slots (off the chain)
                    carry_y = (hists, (NBLK - 1) * SB, ysb,
                               y_d[:, ts0:ts0 + TCk, :])
                else:
                    # final chunk: ship all but the last block early; only
                    # steps 4..7's y work remains after the last update
                    lb0 = (NBLK - 1) * SB
                    if NBLK > 1:
                        nc.sync.dma_start(out=y_d[:, ts0:ts0 + lb0, :],
                                          in_=ysb[:, 0:lb0])
                    emit_y_half(("yf",), lb0, hists, 0, SB // 2, SB)
                    emit_y_half(("yf",), lb0, hists, 1, SB // 2, SB)
                    emit_y_copy(("yf",), ysb, lb0)
                    nc.sync.dma_start(out=y_d[:, ts0 + lb0:ts0 + TCk, :],
                                      in_=ysb[:, lb0:])
                for g in range(G):
                    if ("n0", g) in pvs:
                        carry_pvs[(0, g)] = pvs.pop(("n0", g))
                prev_hists, prev_TC = hists, TCk
    nc.finalize()
    return nc


def get_nc(T=1000, TC=96, SB=8, G=2):
    key = (T, TC, SB, G)
    if key not in _CACHE:
        _CACHE[key] = _build(T, TC, SB, G)
    return _CACHE[key]


def make_inputs(x, noise, W_in, W_rec, W_out_w, W_out_b, bias):
    """Host-side shard + layout prep.  Returns in_maps for 8 cores.

    Exponential rescaling: the device recurrence uses decay d = fp16(0.8),
    slightly below the true 0.8.  Because relu is positively homogeneous,
    running the recurrence on r~_t = c^t r_t with c = d/0.8 (so 0.8*c = d
    exactly), drive scaled by c^(t+1), and the output rescaled by c^-(t+1)
    on the host reproduces the true-decay dynamics exactly.
    """
    x = np.asarray(x, np.float32)
    noise = np.asarray(noise, np.float32)
    W_in = np.asarray(W_in, np.float32)
    W_rec = np.asarray(W_rec, np.float32)
    W_out_w = np.asarray(W_out_w, np.float32)
    bias = np.asarray(bias, np.float32)
    B, T, _ = x.shape

    cfac = DECAY / (1.0 - ALPHA)                       # 0.99975586
    tfac = np.power(cfac, np.arange(1, T + 1), dtype=np.float64).astype(np.float32)

    cpack = np.zeros((128, 774 + 8 * BC), np.float16)  # w4|win|wout|xta blk0
    wrt = ALPHA * cfac * W_rec.T + DECAY * np.eye(256, dtype=np.float32)
    wrt = wrt.astype(np.float16)                       # [k, m]
    for k_c in range(2):
        for m_c in range(2):
            cpack[:, (2 * k_c + m_c) * 128:(2 * k_c + m_c + 1) * 128] = \
                wrt[128 * k_c:128 * (k_c + 1), 128 * m_c:128 * (m_c + 1)]
    ident = np.zeros((128, 128 + 8 * 2 * BC), F8NP)    # I/16 | noise blk0
    ident[:, 0:128] = (np.eye(128, dtype=np.float32) / NOISE_PREMUL).astype(F8NP)
    cpack[:3, 512:768] = (ALPHA * W_in.T).astype(np.float16)
    cpack[3, 512:768] = (ALPHA * bias).astype(np.float16)
    wt = np.asarray(W_out_w, np.float32).T.astype(np.float16)   # [n, 3]
    for k_c in range(2):
        cpack[:, 768 + 3 * k_c:768 + 3 * (k_c + 1)] = \
            wt[128 * k_c:128 * (k_c + 1)]

    nscale = ALPHA * NOISE_SCALE
    in_maps = []
    for c in range(NCORES):
        b0 = c * BC
        nz = (noise[b0:b0 + BC] * (NOISE_PREMUL * nscale * tfac[None, :, None])
              ).astype(F8NP)                           # [32, T, 256]
        nzt = np.ascontiguousarray(
            nz.reshape(BC, T, 2, 128).transpose(3, 1, 2, 0)).reshape(128, T, 2 * BC)
        xc = x[b0:b0 + BC] * tfac[None, :, None]       # [32, T, 3]
        xta = np.empty((4, T, BC), np.float16)
        xta[:3] = xc.transpose(2, 1, 0).astype(np.float16)
        xta[3] = tfac[:, None]
        cpk = cpack.copy()
        cpk[0:4, 774:774 + 8 * BC] = xta[:, 0:8, :].reshape(4, 8 * BC)
        idp = ident.copy()
        idp[:, 128:128 + 8 * 2 * BC] = nzt[:, 0:8, :].reshape(128, 8 * 2 * BC)
        in_maps.append({
            "noiset": nzt, "xta": xta, "cpack": cpk, "ident": idp,
        })
    return in_maps


def gather_output(results, B, T, W_out_b):
    cfac = DECAY / (1.0 - ALPHA)
    inv = np.power(cfac, -np.arange(1, T + 1), dtype=np.float64).astype(np.float32)
    out = np.empty((B, T, 3), np.float32)
    for c in range(NCORES):
        out[c * BC:(c + 1) * BC] = \
            results[c]["y"].transpose(2, 1, 0).astype(np.float32)
    out *= inv[None, :, None]
    out += np.asarray(W_out_b, np.float32)[None, None, :]
    return out


def kernel(x, noise, W_in, W_rec, W_out_w, W_out_b, bias):
    x = np.asarray(x, np.float32)
    B, T, _ = x.shape
    nc = get_nc(T=T)
    in_maps = make_inputs(x, noise, W_in, W_rec, W_out_w, W_out_b, bias)
    res = run_bass_kernel_spmd(nc, in_maps, list(range(NCORES)))
    return gather_output(res.results, B, T, W_out_b)
